# revision 32
# baseline (speedup 1.0000x reference)
"""Multi-head attention (B=4, T=2048, D=1024, H=16) on 8 Trainium2 cores.

Sharding: batch (4-way) x head-half (2-way) -> 8 cores.
Core c handles batch b = c//2 and heads g*8..g*8+8 where g = c%2.

Per-core device program (bf16 matmuls, fp32 psum accumulation):
  Stage A: kT[j,t] (k-part j-tiles) and v[t,j] GEMMs over the resident x^T.
    v is stored as [t, h, 65] with a ones column per head; biases are added
    via K=1 rank-1 matmuls into the psum accumulation groups.
  Stage B (interleaved): for each q j-tile jt, the qT GEMM runs interleaved
    with the attention of heads 2jt, 2jt+1 (the attention is exp/ACT-bound,
    so the qT matmuls fill PE idle slots; their psums share the AV psum
    slots via the pool tag).
  Attention per head: scoresT[ki,qi] = kT2^T qpad (K=128 with the other
    head's rows zeroed in qpad - keeps the PE activity monitor at full
    clock), exp (scale=1/8) straight from psum in 1024-wide ACT ops,
    AV: oT[j,qi] += [v|1|pad]^T wt with M=128 (row 64 = softmax denom).
    AV(k-1) is emitted after scores(k) (software pipeline).
  Normalize: chunk sums copied to 32-aligned partitions, one [128,512]
    parallel reciprocal, DRAM-bounce partition-broadcast, one DVE multiply.
  Out-proj: out[t,c] = ot^T @ woT over 4 j-tiles; output DMAs round-robin
    over two queues.

Host: transposes/reshapes inputs per core (bf16), sums the two head-half
partial outputs per batch, adds out_b.
"""

import numpy as np
import ml_dtypes
from contextlib import ExitStack

import concourse.bass as bass
import concourse.tile as tile
from concourse import bacc, mybir
from concourse.bass_utils import run_bass_kernel_spmd

BF16_NP = ml_dtypes.bfloat16

B, T, D = 4, 2048, 1024
H, HD = 16, 64
P = 128
NC = 8
HPC = 8          # heads per core
JC = HPC * HD    # 512 head-dim columns per core
KT = D // P      # 8 contraction tiles for QKV
TT = T // P      # 16 t tiles
TCH = T // 512   # 4 t chunks of 512
F32 = mybir.dt.float32
BF16 = mybir.dt.bfloat16

_cached = {}


def build_program():
    nc = bacc.Bacc("TRN2", target_bir_lowering=False, debug=False,
                   enable_asserts=True, num_devices=NC)

    xt_d = nc.dram_tensor("xt", [TCH, P, KT, 512], BF16, kind="ExternalInput").ap()
    wqk_d = nc.dram_tensor("wqk", [P, KT, 2 * JC], BF16, kind="ExternalInput").ap()
    wv_d = nc.dram_tensor("wv", [P, KT, JC], BF16, kind="ExternalInput").ap()
    bqk_d = nc.dram_tensor("bqk", [1, 2 * JC], BF16, kind="ExternalInput").ap()
    bv_d = nc.dram_tensor("bv", [1, JC], BF16, kind="ExternalInput").ap()
    ones_d = nc.dram_tensor("ones", [P, 512], BF16, kind="ExternalInput").ap()
    wo_d = nc.dram_tensor("wo", [P, JC // P, D], BF16, kind="ExternalInput").ap()
    out_d = nc.dram_tensor("out", [T, D], F32, kind="ExternalOutput").ap()

    EXP = mybir.ActivationFunctionType.Exp
    VW = HPC * (HD + 1)

    with tile.TileContext(nc) as tc:
        with ExitStack() as ctx:
            persist = ctx.enter_context(tc.tile_pool(name="persist", bufs=1))
            qk_sb = persist.tile([P, 2 * JC // P, T], BF16, tag="qk")
            # [t, 8 heads x [v(64)|ones(1)]] + 64 pad cols so the AV
            # stationary operand can be sliced 128 wide (M=128 keeps the PE
            # activity monitor from downclocking)
            vaug_f = persist.tile([P, TT, VW + HD], BF16, tag="vaug")
            ones2d = persist.tile([P, 512], BF16, tag="ones2d")
            bqk_sb = persist.tile([1, 2 * JC], BF16, tag="bqk")
            bv_sb = persist.tile([1, JC], BF16, tag="bv")
            xt_sb = persist.tile([P, TCH, KT, 512], BF16, tag="xt")
            wqk_sb = persist.tile([P, KT, 2 * JC], BF16, tag="wqk")
            ot_sb = persist.tile([P, JC // P, T], BF16, tag="ot")
            wo_sb = persist.tile([P, JC // P, D], BF16, tag="wo")

            # input DMAs: x chunks on the sync queue, weights on gpsimd.
            # ones/bqk/bv go FIRST - stage A's bias matmuls need them, and
            # behind 4MB of xt they would arrive at ~11.5us
            nc.sync.dma_start(ones2d[:], ones_d[:])
            nc.sync.dma_start(bqk_sb[:], bqk_d[:])
            nc.sync.dma_start(bv_sb[:], bv_d[:])
            for tci in range(TCH):
                nc.sync.dma_start(xt_sb[:, tci], xt_d[tci])
            for k in range(KT):
                nc.gpsimd.dma_start(wqk_sb[:, k, :], wqk_d[:, k, :])
            nc.gpsimd.dma_start(wo_sb[:], wo_d[:])
            ones_sb = ones2d[0:1, :]

            vaug = vaug_f[:, :, 0:VW].rearrange(
                "p t (h e) -> p t h e", h=HPC)          # [128, 16, 8, 65]
            for tt in range(TT):
                nc.vector.tensor_copy(vaug[:, tt, :, HD:HD + 1],
                                      ones2d[:, 0:HPC, None])
                nc.vector.tensor_copy(vaug_f[:, tt, VW:VW + HD],
                                      ones2d[:, 0:HD])

            # ---------------- Stage A: kT and v GEMMs ----------------
            with ExitStack() as c1:
                with nc.named_scope("qkv_a"):
                    wvpool = c1.enter_context(tc.tile_pool(name="wvpool", bufs=1))
                    psA = c1.enter_context(
                        tc.tile_pool(name="psA", bufs=4, space="PSUM"))
                    wv_sb = wvpool.tile([P, KT, JC], BF16, tag="wv")
                    for k in range(KT):
                        nc.gpsimd.dma_start(wv_sb[:, k, :], wv_d[:, k, :])

                    for tci in range(TCH):
                        tsl = slice(tci * 512, (tci + 1) * 512)
                        # kT j-tiles (j = 4..7 of qk_sb)
                        for j in range(4):
                            ps = psA.tile([P, 512], F32, tag="psA",
                                          name=f"psk_{tci}_{j}")
                            for k in range(KT):
                                nc.tensor.matmul(
                                    ps[:],
                                    wqk_sb[:, k, JC + j * P:JC + (j + 1) * P],
                                    xt_sb[:, tci, k, :],
                                    start=(k == 0), stop=False)
                            nc.tensor.matmul(
                                ps[:],
                                bqk_sb[0:1, JC + j * P:JC + (j + 1) * P],
                                ones_sb[0:1, :],
                                start=False, stop=True)
                            nc.vector.tensor_copy(qk_sb[:, 4 + j, tsl], ps[:])
                        # v t-subtiles
                        for tt in range(4):
                            tglob = tci * 4 + tt
                            ps = psA.tile([P, 512], F32, tag="psA",
                                          name=f"psv_{tci}_{tt}")
                            for k in range(KT):
                                nc.tensor.matmul(
                                    ps[:],
                                    xt_sb[:, tci, k, tt * P:(tt + 1) * P],
                                    wv_sb[:, k, :],
                                    start=(k == 0), stop=False)
                            nc.tensor.matmul(
                                ps[:],
                                ones_sb[0:1, 0:P],
                                bv_sb[0:1, :],
                                start=False, stop=True)
                            nc.vector.tensor_copy(
                                vaug[:, tglob, :, 0:HD],
                                ps[:].rearrange("p (h d) -> p h d", h=HPC))

            # ------- Stage B: qT GEMM interleaved with attention -------
            with ExitStack() as c2:
                with nc.named_scope("attn"):
                    wtpool = c2.enter_context(tc.tile_pool(name="wtpool", bufs=3))
                    nrmpool = c2.enter_context(tc.tile_pool(name="nrmpool", bufs=2))
                    rbpool = c2.enter_context(tc.tile_pool(name="rbpool", bufs=2))
                    qpool = c2.enter_context(tc.tile_pool(name="qpool", bufs=2))
                    rdpool = c2.enter_context(
                        tc.tile_pool(name="rdpool", bufs=2, space="DRAM"))
                    pss = c2.enter_context(
                        tc.tile_pool(name="pss", bufs=2, space="PSUM"))
                    psav = c2.enter_context(
                        tc.tile_pool(name="psav", bufs=4, space="PSUM"))

                    # two rotating zero-padded qT buffers; heads alternate
                    # parity so each buffer's zero half stays zero
                    qpads = [qpool.tile([P, T], BF16, tag="qpad",
                                        name=f"qpad_{i}") for i in range(2)]
                    for i in range(2):
                        nc.vector.memset(qpads[i][:], 0.0)

                    def qt_chunk(jt, tci, pool_tag):
                        # one qT GEMM chunk + the next even head's qpad
                        # slice copy (head 2jt reads rows 0-63)
                        tsl = slice(tci * 512, (tci + 1) * 512)
                        if pool_tag == "av":
                            ps = psav.tile([P, 512], F32, tag="av",
                                           name=f"psq_{jt}_{tci}")
                        else:
                            ps = pss.tile([P, 512], F32, tag="ps_s",
                                          name=f"psq_{jt}_{tci}")
                        for k in range(KT):
                            nc.tensor.matmul(
                                ps[:],
                                wqk_sb[:, k, jt * P:(jt + 1) * P],
                                xt_sb[:, tci, k, :],
                                start=(k == 0), stop=False)
                        nc.tensor.matmul(
                            ps[:],
                            bqk_sb[0:1, jt * P:(jt + 1) * P],
                            ones_sb[0:1, :],
                            start=False, stop=True)
                        nc.vector.tensor_copy(qk_sb[:, jt, tsl], ps[:])
                        nc.vector.tensor_copy(
                            qpads[0][0:HD, tsl], qk_sb[0:HD, jt, tsl])

                    def do_scores(h, k, qpad):
                        jt = h // 2
                        # full 128-row stationary operand (both heads' kT);
                        # the other head's rows hit the zero half of qpad, so
                        # the K=128 contraction equals K=64 but keeps the PE
                        # array fully active (HAM stays at 8/8)
                        kT2 = qk_sb[:, jt + 4, :]
                        wt = wtpool.tile([P, T], BF16, tag="wt",
                                         name=f"wt_{h}_{k}")
                        for half in range(2):
                            ps = pss.tile([P, 2, 512], F32, tag="ps_s",
                                          name=f"ps_s_{h}_{k}_{half}")
                            for cc in range(2):
                                c4 = half * 2 + cc
                                nc.tensor.matmul(
                                    ps[:, cc, :],
                                    kT2[:, k * P:(k + 1) * P],
                                    qpad[:, c4 * 512:(c4 + 1) * 512],
                                    start=True, stop=True)
                            nc.scalar.activation(
                                wt[:, half * 1024:(half + 1) * 1024],
                                ps[:].rearrange("p a b -> p (a b)"),
                                EXP, bias=0.0, scale=0.125)
                        return wt

                    def do_av(h, k, wt, av_tiles):
                        for c4 in range(4):
                            nc.tensor.matmul(
                                av_tiles[c4][:],
                                vaug_f[:, k, h * (HD + 1):h * (HD + 1) + P],
                                wt[:, c4 * 512:(c4 + 1) * 512],
                                start=(k == 0), stop=(k == TT - 1))

                    def finish_head(h, av_tiles):
                        pb = (h % 2) * 64
                        jt = h // 2
                        # free psum fast: copy o rows (unnormalized); chunk
                        # sums go to 32-aligned partitions so one [128,512]
                        # reciprocal covers all four chunks on parallel lanes
                        sums = nrmpool.tile([P, 512], F32, tag="sums",
                                            name=f"sums_{h}")
                        for c4 in range(4):
                            csl = slice(c4 * 512, (c4 + 1) * 512)
                            nc.vector.tensor_copy(
                                ot_sb[pb:pb + 64, jt, csl],
                                av_tiles[c4][0:HD, :])
                            nc.vector.tensor_copy(
                                sums[32 * c4:32 * c4 + 1, :],
                                av_tiles[c4][HD:HD + 1, :])
                        rcp = nrmpool.tile([P, 512], F32, tag="rcp",
                                           name=f"rcp_{h}")
                        nc.vector.reciprocal(rcp[:], sums[:])
                        rd = rdpool.tile([4, 512], F32, tag="rd",
                                         name=f"rd_{h}")
                        nc.sync.dma_start(rd[:], rcp[0:128:32, :])
                        rb = rbpool.tile([P, T], F32, tag="rb",
                                         name=f"rb_{h}")
                        rd_bcast = bass.AP(
                            tensor=rd.tensor, offset=rd.offset,
                            ap=[[0, 64], [512, 4], [1, 512]])
                        nc.sync.dma_start(
                            rb[pb:pb + 64, :].rearrange(
                                "p (c r) -> p c r", c=4),
                            rd_bcast)
                        nc.vector.tensor_mul(
                            ot_sb[pb:pb + 64, jt, :],
                            ot_sb[pb:pb + 64, jt, :],
                            rb[pb:pb + 64, :])

                    # software pipeline: AV(k-1) emitted after scores(k).
                    # qT for j-tile jt>=1 is emitted in the back half of the
                    # previous odd head (k=9,11,13,15) with per-chunk qpad
                    # copies, so head-pair boundaries keep the exp stream
                    # hot; odd heads' qpad slices prefetch at the same spots.
                    for tci in range(TCH):
                        qt_chunk(0, tci, "av")
                    prev = None
                    for h in range(HPC):
                        pb = (h % 2) * 64
                        jt = h // 2
                        qpad = qpads[h % 2]
                        av_tiles = [psav.tile([P, 512], F32, tag="av",
                                              name=f"av_{h}_{i}")
                                    for i in range(4)]
                        for k in range(TT):
                            wt = do_scores(h, k, qpad)
                            if k in (9, 11, 13, 15):
                                idx = (k - 9) // 2
                                tsl = slice(idx * 512, (idx + 1) * 512)
                                if h % 2 == 1 and jt + 1 < 4:
                                    qt_chunk(jt + 1, idx, "ps_s")
                                elif h % 2 == 0:
                                    nc.vector.tensor_copy(
                                        qpads[1][64:128, tsl],
                                        qk_sb[64:128, jt, tsl])
                            if prev is not None:
                                ph, pk, pwt, pav = prev
                                do_av(ph, pk, pwt, pav)
                                if pk == TT - 1:
                                    finish_head(ph, pav)
                            prev = (h, k, wt, av_tiles)
                    ph, pk, pwt, pav = prev
                    do_av(ph, pk, pwt, pav)
                    # last head: kick the reciprocal/DMA-bounce chain off
                    # before the o-copies so it overlaps them
                    sums7 = nrmpool.tile([P, 512], F32, tag="sums",
                                         name="sums_7f")
                    for c4 in range(4):
                        nc.vector.tensor_copy(
                            sums7[32 * c4:32 * c4 + 1, :],
                            pav[c4][HD:HD + 1, :])
                    rcp7 = nrmpool.tile([P, 512], F32, tag="rcp",
                                        name="rcp_7f")
                    nc.vector.reciprocal(rcp7[:], sums7[:])
                    rd7 = rdpool.tile([4, 512], F32, tag="rd", name="rd_7f")
                    nc.sync.dma_start(rd7[:], rcp7[0:128:32, :])
                    rb7 = rbpool.tile([P, T], F32, tag="rb", name="rb_7f")
                    rd7_bcast = bass.AP(
                        tensor=rd7.tensor, offset=rd7.offset,
                        ap=[[0, 64], [512, 4], [1, 512]])
                    nc.sync.dma_start(
                        rb7[64:128, :].rearrange("p (c r) -> p c r", c=4),
                        rd7_bcast)
                    for c4 in range(4):
                        csl = slice(c4 * 512, (c4 + 1) * 512)
                        nc.vector.tensor_copy(
                            ot_sb[64:128, 3, csl], pav[c4][0:HD, :])
                    nc.vector.tensor_mul(
                        ot_sb[64:128, 3, :], ot_sb[64:128, 3, :],
                        rb7[64:128, :])

            # ---------------- Phase 3: out projection ----------------
            with ExitStack() as c3:
                with nc.named_scope("outproj"):
                    opool = c3.enter_context(tc.tile_pool(name="opool", bufs=3))
                    ps3 = c3.enter_context(
                        tc.tile_pool(name="ps3", bufs=4, space="PSUM"))

                    # front-run jt0-2 of the first four psum groups (no
                    # head-7 dependency) while finish_head(7) drains
                    front = {}
                    for tt in range(2):
                        for cc in range(2):
                            ps = ps3.tile([P, 512], F32, tag="pso",
                                          name=f"pso_f_{tt}_{cc}")
                            for jt in range(3):
                                nc.tensor.matmul(
                                    ps[:],
                                    ot_sb[:, jt, tt * P:(tt + 1) * P],
                                    wo_sb[:, jt, cc * 512:(cc + 1) * 512],
                                    start=(jt == 0), stop=False)
                            front[(tt, cc)] = ps
                    for tt in range(TT):
                        ost = opool.tile([P, D], F32, tag="ost")
                        for cc in range(2):
                            if tt < 2:
                                ps = front[(tt, cc)]
                                nc.tensor.matmul(
                                    ps[:],
                                    ot_sb[:, 3, tt * P:(tt + 1) * P],
                                    wo_sb[:, 3, cc * 512:(cc + 1) * 512],
                                    start=False, stop=True)
                            else:
                                ps = ps3.tile([P, 512], F32, tag="pso",
                                              name=f"pso_{tt}_{cc}")
                                for jt in range(JC // P):
                                    nc.tensor.matmul(
                                        ps[:],
                                        ot_sb[:, jt, tt * P:(tt + 1) * P],
                                        wo_sb[:, jt,
                                              cc * 512:(cc + 1) * 512],
                                        start=(jt == 0),
                                        stop=(jt == JC // P - 1))
                            nc.vector.tensor_copy(
                                ost[:, cc * 512:(cc + 1) * 512], ps[:])
                        eng = nc.sync if tt % 2 == 0 else nc.gpsimd
                        eng.dma_start(out_d[tt * P:(tt + 1) * P, :], ost[:])

    nc.compile()
    return nc


def _prep_core_inputs(x, qkv_w, qkv_b, out_w, core):
    b, g = core // 2, core % 2
    jsl = slice(g * JC, (g + 1) * JC)

    xT = np.ascontiguousarray(x[b].T)                       # [1024, 2048]
    xt = np.ascontiguousarray(
        xT.reshape(KT, P, TCH, 512).transpose(2, 1, 0, 3))  # [4, 128, 8, 512]

    Wq = qkv_w[0 * D:1 * D][jsl]                            # [512, 1024]
    Wk = qkv_w[1 * D:2 * D][jsl]
    Wv = qkv_w[2 * D:3 * D][jsl]
    WqkT = np.concatenate([Wq, Wk], axis=0).T               # [1024, 1024]
    wqk = np.ascontiguousarray(
        WqkT.reshape(KT, P, 2 * JC).transpose(1, 0, 2))     # [128, 8, 1024]
    WvT = Wv.T                                              # [1024, 512]
    wv = np.ascontiguousarray(
        WvT.reshape(KT, P, JC).transpose(1, 0, 2))          # [128, 8, 512]

    bqk = np.concatenate(
        [qkv_b[0 * D:1 * D][jsl], qkv_b[1 * D:2 * D][jsl]])[None, :]
    bv = qkv_b[2 * D:3 * D][jsl][None, :]

    WoT = np.ascontiguousarray(out_w[:, jsl].T)             # [512, 1024]
    wo = np.ascontiguousarray(
        WoT.reshape(JC // P, P, D).transpose(1, 0, 2))      # [128, 4, 1024]

    return {
        "xt": xt.astype(BF16_NP),
        "wqk": wqk.astype(BF16_NP),
        "wv": wv.astype(BF16_NP),
        "bqk": bqk.astype(BF16_NP),
        "bv": bv.astype(BF16_NP),
        "wo": wo.astype(BF16_NP),
        "ones": np.ones((P, 512), dtype=BF16_NP),
    }


def run(x, qkv_w, qkv_b, out_w, out_b, trace=False, tmpdir=None):
    if "nc" not in _cached:
        _cached["nc"] = build_program()
    nc = _cached["nc"]
    in_maps = [_prep_core_inputs(x, qkv_w, qkv_b, out_w, c) for c in range(NC)]
    res = run_bass_kernel_spmd(nc, in_maps, core_ids=list(range(NC)),
                               trace=trace, tmpdir=tmpdir)
    parts = np.stack([res.results[c]["out"] for c in range(NC)])  # [8, T, D]
    out = parts.reshape(B, 2, T, D).sum(axis=1) + out_b[None, None, :]
    return out.astype(np.float32), res


def kernel(x, qkv_w, qkv_b, out_w, out_b):
    x = np.asarray(x, dtype=np.float32)
    qkv_w = np.asarray(qkv_w, dtype=np.float32)
    qkv_b = np.asarray(qkv_b, dtype=np.float32)
    out_w = np.asarray(out_w, dtype=np.float32)
    out_b = np.asarray(out_b, dtype=np.float32)
    out, _ = run(x, qkv_w, qkv_b, out_w, out_b, trace=False)
    return out



# revision 33
# speedup vs baseline: 1.0112x; 1.0112x over previous
"""Multi-head attention (B=4, T=2048, D=1024, H=16) on 8 Trainium2 cores.

Sharding: batch (4-way) x head-half (2-way) -> 8 cores.
Core c handles batch b = c//2 and heads g*8..g*8+8 where g = c%2.

Per-core device program (bf16 matmuls, fp32 psum accumulation):
  Stage A: kT[j,t] (k-part j-tiles) and v[t,j] GEMMs over the resident x^T.
    v is stored as [t, h, 65] with a ones column per head; biases are added
    via K=1 rank-1 matmuls into the psum accumulation groups.
  Stage B (interleaved): for each q j-tile jt, the qT GEMM runs interleaved
    with the attention of heads 2jt, 2jt+1 (the attention is exp/ACT-bound,
    so the qT matmuls fill PE idle slots; their psums share the AV psum
    slots via the pool tag).
  Attention per head: scoresT[ki,qi] = kT2^T qpad (K=128 with the other
    head's rows zeroed in qpad - keeps the PE activity monitor at full
    clock), exp (scale=1/8) straight from psum in 1024-wide ACT ops,
    AV: oT[j,qi] += [v|1|pad]^T wt with M=128 (row 64 = softmax denom).
    AV(k-1) is emitted after scores(k) (software pipeline).
  Normalize: chunk sums copied to 32-aligned partitions, one [128,512]
    parallel reciprocal, DRAM-bounce partition-broadcast, one DVE multiply.
  Out-proj: out[t,c] = ot^T @ woT over 4 j-tiles; output DMAs round-robin
    over two queues.

Host: transposes/reshapes inputs per core (bf16), sums the two head-half
partial outputs per batch, adds out_b.
"""

import numpy as np
import ml_dtypes
from contextlib import ExitStack

import concourse.bass as bass
import concourse.tile as tile
from concourse import bacc, mybir
from concourse.bass_utils import run_bass_kernel_spmd

BF16_NP = ml_dtypes.bfloat16

B, T, D = 4, 2048, 1024
H, HD = 16, 64
P = 128
NC = 8
HPC = 8          # heads per core
JC = HPC * HD    # 512 head-dim columns per core
KT = D // P      # 8 contraction tiles for QKV
TT = T // P      # 16 t tiles
TCH = T // 512   # 4 t chunks of 512
F32 = mybir.dt.float32
BF16 = mybir.dt.bfloat16

_cached = {}


def build_program():
    nc = bacc.Bacc("TRN2", target_bir_lowering=False, debug=False,
                   enable_asserts=True, num_devices=NC)

    xt_d = nc.dram_tensor("xt", [TCH, P, KT, 512], BF16, kind="ExternalInput").ap()
    wqk_d = nc.dram_tensor("wqk", [P, KT, 2 * JC], BF16, kind="ExternalInput").ap()
    wv_d = nc.dram_tensor("wv", [P, KT, JC], BF16, kind="ExternalInput").ap()
    bqk_d = nc.dram_tensor("bqk", [1, 2 * JC], BF16, kind="ExternalInput").ap()
    bv_d = nc.dram_tensor("bv", [1, JC], BF16, kind="ExternalInput").ap()
    ones_d = nc.dram_tensor("ones", [P, 512], BF16, kind="ExternalInput").ap()
    wo_d = nc.dram_tensor("wo", [P, JC // P, D], BF16, kind="ExternalInput").ap()
    out_d = nc.dram_tensor("out", [T, D], F32, kind="ExternalOutput").ap()

    EXP = mybir.ActivationFunctionType.Exp
    VW = HPC * (HD + 1)

    with tile.TileContext(nc) as tc:
        with ExitStack() as ctx:
            persist = ctx.enter_context(tc.tile_pool(name="persist", bufs=1))
            qk_sb = persist.tile([P, 2 * JC // P, T], BF16, tag="qk")
            # [t, 8 heads x [v(64)|ones(1)]] + 64 pad cols so the AV
            # stationary operand can be sliced 128 wide (M=128 keeps the PE
            # activity monitor from downclocking)
            vaug_f = persist.tile([P, TT, VW + HD], BF16, tag="vaug")
            ones2d = persist.tile([P, 512], BF16, tag="ones2d")
            bqk_sb = persist.tile([1, 2 * JC], BF16, tag="bqk")
            bv_sb = persist.tile([1, JC], BF16, tag="bv")
            xt_sb = persist.tile([P, TCH, KT, 512], BF16, tag="xt")
            wqk_sb = persist.tile([P, KT, 2 * JC], BF16, tag="wqk")
            ot_sb = persist.tile([P, JC // P, T], BF16, tag="ot")
            wo_sb = persist.tile([P, JC // P, D], BF16, tag="wo")

            # input DMAs: x chunks on the sync queue, weights on gpsimd
            for tci in range(TCH):
                nc.sync.dma_start(xt_sb[:, tci], xt_d[tci])
            for k in range(KT):
                nc.gpsimd.dma_start(wqk_sb[:, k, :], wqk_d[:, k, :])
            nc.sync.dma_start(ones2d[:], ones_d[:])
            nc.sync.dma_start(bqk_sb[:], bqk_d[:])
            nc.sync.dma_start(bv_sb[:], bv_d[:])
            nc.gpsimd.dma_start(wo_sb[:], wo_d[:])
            ones_sb = ones2d[0:1, :]

            vaug = vaug_f[:, :, 0:VW].rearrange(
                "p t (h e) -> p t h e", h=HPC)          # [128, 16, 8, 65]
            for tt in range(TT):
                nc.vector.tensor_copy(vaug[:, tt, :, HD:HD + 1],
                                      ones2d[:, 0:HPC, None])
                nc.vector.tensor_copy(vaug_f[:, tt, VW:VW + HD],
                                      ones2d[:, 0:HD])

            # ---------------- Stage A: kT and v GEMMs ----------------
            with ExitStack() as c1:
                with nc.named_scope("qkv_a"):
                    wvpool = c1.enter_context(tc.tile_pool(name="wvpool", bufs=1))
                    psA = c1.enter_context(
                        tc.tile_pool(name="psA", bufs=4, space="PSUM"))
                    wv_sb = wvpool.tile([P, KT, JC], BF16, tag="wv")
                    for k in range(KT):
                        nc.gpsimd.dma_start(wv_sb[:, k, :], wv_d[:, k, :])

                    for tci in range(TCH):
                        tsl = slice(tci * 512, (tci + 1) * 512)
                        # kT j-tiles (j = 4..7 of qk_sb)
                        for j in range(4):
                            ps = psA.tile([P, 512], F32, tag="psA",
                                          name=f"psk_{tci}_{j}")
                            for k in range(KT):
                                nc.tensor.matmul(
                                    ps[:],
                                    wqk_sb[:, k, JC + j * P:JC + (j + 1) * P],
                                    xt_sb[:, tci, k, :],
                                    start=(k == 0), stop=False)
                            nc.tensor.matmul(
                                ps[:],
                                bqk_sb[0:1, JC + j * P:JC + (j + 1) * P],
                                ones_sb[0:1, :],
                                start=False, stop=True)
                            nc.vector.tensor_copy(qk_sb[:, 4 + j, tsl], ps[:])
                        # v t-subtiles
                        for tt in range(4):
                            tglob = tci * 4 + tt
                            ps = psA.tile([P, 512], F32, tag="psA",
                                          name=f"psv_{tci}_{tt}")
                            for k in range(KT):
                                nc.tensor.matmul(
                                    ps[:],
                                    xt_sb[:, tci, k, tt * P:(tt + 1) * P],
                                    wv_sb[:, k, :],
                                    start=(k == 0), stop=False)
                            nc.tensor.matmul(
                                ps[:],
                                ones_sb[0:1, 0:P],
                                bv_sb[0:1, :],
                                start=False, stop=True)
                            nc.vector.tensor_copy(
                                vaug[:, tglob, :, 0:HD],
                                ps[:].rearrange("p (h d) -> p h d", h=HPC))

            # ------- Stage B: qT GEMM interleaved with attention -------
            with ExitStack() as c2:
                with nc.named_scope("attn"):
                    wtpool = c2.enter_context(tc.tile_pool(name="wtpool", bufs=3))
                    nrmpool = c2.enter_context(tc.tile_pool(name="nrmpool", bufs=2))
                    rbpool = c2.enter_context(tc.tile_pool(name="rbpool", bufs=2))
                    qpool = c2.enter_context(tc.tile_pool(name="qpool", bufs=2))
                    rdpool = c2.enter_context(
                        tc.tile_pool(name="rdpool", bufs=2, space="DRAM"))
                    pss = c2.enter_context(
                        tc.tile_pool(name="pss", bufs=2, space="PSUM"))
                    psav = c2.enter_context(
                        tc.tile_pool(name="psav", bufs=4, space="PSUM"))

                    # two rotating zero-padded qT buffers; heads alternate
                    # parity so each buffer's zero half stays zero
                    qpads = [qpool.tile([P, T], BF16, tag="qpad",
                                        name=f"qpad_{i}") for i in range(2)]
                    for i in range(2):
                        nc.vector.memset(qpads[i][:], 0.0)

                    def qt_chunk(jt, tci, pool_tag):
                        # one qT GEMM chunk + the next even head's qpad
                        # slice copy (head 2jt reads rows 0-63)
                        tsl = slice(tci * 512, (tci + 1) * 512)
                        if pool_tag == "av":
                            ps = psav.tile([P, 512], F32, tag="av",
                                           name=f"psq_{jt}_{tci}")
                        else:
                            ps = pss.tile([P, 512], F32, tag="ps_s",
                                          name=f"psq_{jt}_{tci}")
                        for k in range(KT):
                            nc.tensor.matmul(
                                ps[:],
                                wqk_sb[:, k, jt * P:(jt + 1) * P],
                                xt_sb[:, tci, k, :],
                                start=(k == 0), stop=False)
                        nc.tensor.matmul(
                            ps[:],
                            bqk_sb[0:1, jt * P:(jt + 1) * P],
                            ones_sb[0:1, :],
                            start=False, stop=True)
                        nc.vector.tensor_copy(qk_sb[:, jt, tsl], ps[:])
                        nc.vector.tensor_copy(
                            qpads[0][0:HD, tsl], qk_sb[0:HD, jt, tsl])

                    def do_scores(h, k, qpad):
                        jt = h // 2
                        # full 128-row stationary operand (both heads' kT);
                        # the other head's rows hit the zero half of qpad, so
                        # the K=128 contraction equals K=64 but keeps the PE
                        # array fully active (HAM stays at 8/8)
                        kT2 = qk_sb[:, jt + 4, :]
                        wt = wtpool.tile([P, T], BF16, tag="wt",
                                         name=f"wt_{h}_{k}")
                        for half in range(2):
                            ps = pss.tile([P, 2, 512], F32, tag="ps_s",
                                          name=f"ps_s_{h}_{k}_{half}")
                            for cc in range(2):
                                c4 = half * 2 + cc
                                nc.tensor.matmul(
                                    ps[:, cc, :],
                                    kT2[:, k * P:(k + 1) * P],
                                    qpad[:, c4 * 512:(c4 + 1) * 512],
                                    start=True, stop=True)
                            nc.scalar.activation(
                                wt[:, half * 1024:(half + 1) * 1024],
                                ps[:].rearrange("p a b -> p (a b)"),
                                EXP, bias=0.0, scale=0.125)
                        return wt

                    def do_av(h, k, wt, av_tiles):
                        for c4 in range(4):
                            nc.tensor.matmul(
                                av_tiles[c4][:],
                                vaug_f[:, k, h * (HD + 1):h * (HD + 1) + P],
                                wt[:, c4 * 512:(c4 + 1) * 512],
                                start=(k == 0), stop=(k == TT - 1))

                    def finish_head(h, av_tiles):
                        pb = (h % 2) * 64
                        jt = h // 2
                        # free psum fast: copy o rows (unnormalized); chunk
                        # sums go to 32-aligned partitions so one [128,512]
                        # reciprocal covers all four chunks on parallel lanes
                        sums = nrmpool.tile([P, 512], F32, tag="sums",
                                            name=f"sums_{h}")
                        for c4 in range(4):
                            csl = slice(c4 * 512, (c4 + 1) * 512)
                            nc.vector.tensor_copy(
                                ot_sb[pb:pb + 64, jt, csl],
                                av_tiles[c4][0:HD, :])
                            nc.vector.tensor_copy(
                                sums[32 * c4:32 * c4 + 1, :],
                                av_tiles[c4][HD:HD + 1, :])
                        rcp = nrmpool.tile([P, 512], F32, tag="rcp",
                                           name=f"rcp_{h}")
                        nc.vector.reciprocal(rcp[:], sums[:])
                        rd = rdpool.tile([4, 512], F32, tag="rd",
                                         name=f"rd_{h}")
                        nc.sync.dma_start(rd[:], rcp[0:128:32, :])
                        rb = rbpool.tile([P, T], F32, tag="rb",
                                         name=f"rb_{h}")
                        rd_bcast = bass.AP(
                            tensor=rd.tensor, offset=rd.offset,
                            ap=[[0, 64], [512, 4], [1, 512]])
                        nc.sync.dma_start(
                            rb[pb:pb + 64, :].rearrange(
                                "p (c r) -> p c r", c=4),
                            rd_bcast)
                        nc.vector.tensor_mul(
                            ot_sb[pb:pb + 64, jt, :],
                            ot_sb[pb:pb + 64, jt, :],
                            rb[pb:pb + 64, :])

                    # software pipeline: AV(k-1) emitted after scores(k).
                    # qT for j-tile jt>=1 is emitted in the back half of the
                    # previous odd head (k=9,11,13,15) with per-chunk qpad
                    # copies, so head-pair boundaries keep the exp stream
                    # hot; odd heads' qpad slices prefetch at the same spots.
                    for tci in range(TCH):
                        qt_chunk(0, tci, "av")
                    prev = None
                    for h in range(HPC):
                        pb = (h % 2) * 64
                        jt = h // 2
                        qpad = qpads[h % 2]
                        av_tiles = [psav.tile([P, 512], F32, tag="av",
                                              name=f"av_{h}_{i}")
                                    for i in range(4)]
                        for k in range(TT):
                            wt = do_scores(h, k, qpad)
                            if k in (9, 11, 13, 15):
                                idx = (k - 9) // 2
                                tsl = slice(idx * 512, (idx + 1) * 512)
                                if h % 2 == 1 and jt + 1 < 4:
                                    qt_chunk(jt + 1, idx, "ps_s")
                                elif h % 2 == 0:
                                    nc.vector.tensor_copy(
                                        qpads[1][64:128, tsl],
                                        qk_sb[64:128, jt, tsl])
                            if prev is not None:
                                ph, pk, pwt, pav = prev
                                do_av(ph, pk, pwt, pav)
                                if pk == TT - 1:
                                    finish_head(ph, pav)
                            prev = (h, k, wt, av_tiles)
                    ph, pk, pwt, pav = prev
                    do_av(ph, pk, pwt, pav)
                    # last head: kick the reciprocal/DMA-bounce chain off
                    # before the o-copies so it overlaps them
                    sums7 = nrmpool.tile([P, 512], F32, tag="sums",
                                         name="sums_7f")
                    for c4 in range(4):
                        nc.vector.tensor_copy(
                            sums7[32 * c4:32 * c4 + 1, :],
                            pav[c4][HD:HD + 1, :])
                    rcp7 = nrmpool.tile([P, 512], F32, tag="rcp",
                                        name="rcp_7f")
                    nc.vector.reciprocal(rcp7[:], sums7[:])
                    rd7 = rdpool.tile([4, 512], F32, tag="rd", name="rd_7f")
                    nc.sync.dma_start(rd7[:], rcp7[0:128:32, :])
                    rb7 = rbpool.tile([P, T], F32, tag="rb", name="rb_7f")
                    rd7_bcast = bass.AP(
                        tensor=rd7.tensor, offset=rd7.offset,
                        ap=[[0, 64], [512, 4], [1, 512]])
                    nc.sync.dma_start(
                        rb7[64:128, :].rearrange("p (c r) -> p c r", c=4),
                        rd7_bcast)
                    for c4 in range(4):
                        csl = slice(c4 * 512, (c4 + 1) * 512)
                        nc.vector.tensor_copy(
                            ot_sb[64:128, 3, csl], pav[c4][0:HD, :])
                    nc.vector.tensor_mul(
                        ot_sb[64:128, 3, :], ot_sb[64:128, 3, :],
                        rb7[64:128, :])

            # ---------------- Phase 3: out projection ----------------
            with ExitStack() as c3:
                with nc.named_scope("outproj"):
                    opool = c3.enter_context(tc.tile_pool(name="opool", bufs=4))
                    ps3 = c3.enter_context(
                        tc.tile_pool(name="ps3", bufs=8, space="PSUM"))

                    # front-run jt0-2 of the first four psum groups (no
                    # head-7 dependency) while finish_head(7) drains
                    front = {}
                    for tt in range(4):
                        for cc in range(2):
                            ps = ps3.tile([P, 512], F32, tag="pso",
                                          name=f"pso_f_{tt}_{cc}")
                            for jt in range(3):
                                nc.tensor.matmul(
                                    ps[:],
                                    ot_sb[:, jt, tt * P:(tt + 1) * P],
                                    wo_sb[:, jt, cc * 512:(cc + 1) * 512],
                                    start=(jt == 0), stop=False)
                            front[(tt, cc)] = ps
                    for tt in range(TT):
                        ost = opool.tile([P, D], F32, tag="ost")
                        for cc in range(2):
                            if tt < 4:
                                ps = front[(tt, cc)]
                                nc.tensor.matmul(
                                    ps[:],
                                    ot_sb[:, 3, tt * P:(tt + 1) * P],
                                    wo_sb[:, 3, cc * 512:(cc + 1) * 512],
                                    start=False, stop=True)
                            else:
                                ps = ps3.tile([P, 512], F32, tag="pso",
                                              name=f"pso_{tt}_{cc}")
                                for jt in range(JC // P):
                                    nc.tensor.matmul(
                                        ps[:],
                                        ot_sb[:, jt, tt * P:(tt + 1) * P],
                                        wo_sb[:, jt,
                                              cc * 512:(cc + 1) * 512],
                                        start=(jt == 0),
                                        stop=(jt == JC // P - 1))
                            nc.vector.tensor_copy(
                                ost[:, cc * 512:(cc + 1) * 512], ps[:])
                        eng = nc.sync if tt % 2 == 0 else nc.gpsimd
                        eng.dma_start(out_d[tt * P:(tt + 1) * P, :], ost[:])

    nc.compile()
    return nc


def _prep_core_inputs(x, qkv_w, qkv_b, out_w, core):
    b, g = core // 2, core % 2
    jsl = slice(g * JC, (g + 1) * JC)

    xT = np.ascontiguousarray(x[b].T)                       # [1024, 2048]
    xt = np.ascontiguousarray(
        xT.reshape(KT, P, TCH, 512).transpose(2, 1, 0, 3))  # [4, 128, 8, 512]

    Wq = qkv_w[0 * D:1 * D][jsl]                            # [512, 1024]
    Wk = qkv_w[1 * D:2 * D][jsl]
    Wv = qkv_w[2 * D:3 * D][jsl]
    WqkT = np.concatenate([Wq, Wk], axis=0).T               # [1024, 1024]
    wqk = np.ascontiguousarray(
        WqkT.reshape(KT, P, 2 * JC).transpose(1, 0, 2))     # [128, 8, 1024]
    WvT = Wv.T                                              # [1024, 512]
    wv = np.ascontiguousarray(
        WvT.reshape(KT, P, JC).transpose(1, 0, 2))          # [128, 8, 512]

    bqk = np.concatenate(
        [qkv_b[0 * D:1 * D][jsl], qkv_b[1 * D:2 * D][jsl]])[None, :]
    bv = qkv_b[2 * D:3 * D][jsl][None, :]

    WoT = np.ascontiguousarray(out_w[:, jsl].T)             # [512, 1024]
    wo = np.ascontiguousarray(
        WoT.reshape(JC // P, P, D).transpose(1, 0, 2))      # [128, 4, 1024]

    return {
        "xt": xt.astype(BF16_NP),
        "wqk": wqk.astype(BF16_NP),
        "wv": wv.astype(BF16_NP),
        "bqk": bqk.astype(BF16_NP),
        "bv": bv.astype(BF16_NP),
        "wo": wo.astype(BF16_NP),
        "ones": np.ones((P, 512), dtype=BF16_NP),
    }


def run(x, qkv_w, qkv_b, out_w, out_b, trace=False, tmpdir=None):
    if "nc" not in _cached:
        _cached["nc"] = build_program()
    nc = _cached["nc"]
    in_maps = [_prep_core_inputs(x, qkv_w, qkv_b, out_w, c) for c in range(NC)]
    res = run_bass_kernel_spmd(nc, in_maps, core_ids=list(range(NC)),
                               trace=trace, tmpdir=tmpdir)
    parts = np.stack([res.results[c]["out"] for c in range(NC)])  # [8, T, D]
    out = parts.reshape(B, 2, T, D).sum(axis=1) + out_b[None, None, :]
    return out.astype(np.float32), res


def kernel(x, qkv_w, qkv_b, out_w, out_b):
    x = np.asarray(x, dtype=np.float32)
    qkv_w = np.asarray(qkv_w, dtype=np.float32)
    qkv_b = np.asarray(qkv_b, dtype=np.float32)
    out_w = np.asarray(out_w, dtype=np.float32)
    out_b = np.asarray(out_b, dtype=np.float32)
    out, _ = run(x, qkv_w, qkv_b, out_w, out_b, trace=False)
    return out

